# revision 4
# baseline (speedup 1.0000x reference)
"""Multi-head attention (B=2, S=4096, D=768, H=12) on 8 Trainium2 cores.

Sharding: core c -> batch b = c // 4, head-triple g = c % 4 (heads 3g..3g+2).
Each core computes its QKV projections (columns of W for its heads) and
flash-style attention for its 3 heads, fully on-chip; no cross-core comms.

v3 design (HW-microbenched matmul patterns + concurrent exp drain):
  - scores: K=64 row-tiled kc-PAIRS -- even kc on PE rows 0:64 (tile (0,0)),
    odd kc on rows 64:128 (tile (64,0)); concurrent subarray execution writes
    two PSUM banks at once (~63 ns/MM bf16 vs 428 ns/MM solo K=64). qT/kT are
    bf16, replicated on both partition halves to feed both tiles.
  - PV: contraction split 128 -> 2x64 row-tiled strips accumulating into two
    PSUM banks (~78 ns/MM fp16 vs 213 for K=128); halves merged for free by
    accumulating PE transposes in the finalize.
  - exp drain runs on TWO engines concurrently: most kc-pair groups use
    ScalarE exact exp; a fixed subset of group indices (same kc's in every
    iteration) use an fp16 Schraudolph-product approximation on DVE (int16
    affine from PSUM + phase-shifted bitcast product, ~0.9% rms on those
    kc's). Its constant scale C is folded into the mask path host-side
    (mask += ln(1/C)/1e4 for those kc columns -> em picks up 1/C -> vE and
    the ones-column denominator are both pre-scaled; softmax stays exact in
    expectation). Concurrency matters: the scores PSUM pool is only 3 groups
    deep, so a single engine's drain rate lower-bounds the pipeline.
"""

import os
import sys

if "/opt/trn_rl_repo" not in sys.path:
    sys.path.insert(0, "/opt/trn_rl_repo")

from contextlib import ExitStack

import numpy as np

import concourse.bass as bass
import concourse.tile as tile
from concourse import bacc, mybir
from concourse.bass_utils import run_bass_kernel_spmd
from concourse.masks import make_identity

F32 = mybir.dt.float32
F16 = mybir.dt.float16
BF16 = mybir.dt.bfloat16
I16 = mybir.dt.int16
AF = mybir.ActivationFunctionType
ALU = mybir.AluOpType
F16_NP = np.float16

B, S, D, H, DK = 2, 4096, 768, 12, 64
N_CORES = 8
HPG = 3            # heads per core
GD = HPG * DK      # 192 output columns per core
SQ = 512           # q-chunk width
NSQ = S // SQ      # 8
KCW = 128          # k-chunk width
NKC = S // KCW     # 32
GRP = 2            # k-chunks per exp group (kc pair -> 2 PSUM banks)
NG = NKC // GRP    # 16 groups per (h, sq)
NDC = D // 128     # 6 contraction chunks
QTR = S // 4       # transpose/projection pipeline granularity
SQQ = NSQ // 4     # q chunks per quarter
SCQ = NKC // 4     # s chunks per quarter

# Schraudolph fp16 product-exp: p = bc16(i1)*bc16(i2), i1 = rne(A*s + B1),
# i2 = i1 - 512  ~=  C * exp(s/8). C (measured over N(0, 2.46^2) scores) is
# folded into the mask for the DVE kc columns host-side.
A_EXP = 1024.0 / (16.0 * 0.6931471805599453)
B1_EXP = 15616.0
C_EXP = 1.0830154


def dve_gi_set():
    n = int(os.environ.get("BASS_NDVE", "5"))
    if n <= 0:
        return set()
    # spread across 2..15 (keep g0/g1 on ScalarE for pipeline warmup)
    lo, hi = 2, NG
    return {lo + (i * (hi - lo)) // n for i in range(n)}


def _emit(ctx: ExitStack, tc: tile.TileContext, io: dict):
    nc = tc.nc
    dve_gi = dve_gi_set()

    const = ctx.enter_context(tc.tile_pool(name="const", bufs=1))
    xt_pool = ctx.enter_context(tc.tile_pool(name="xt", bufs=5))
    proj = ctx.enter_context(tc.tile_pool(name="proj", bufs=1))
    scores_pool = ctx.enter_context(tc.tile_pool(name="scores", bufs=3, space="PSUM"))
    pv_pool = ctx.enter_context(tc.tile_pool(name="pvp", bufs=1, space="PSUM"))
    probs_pool = ctx.enter_context(tc.tile_pool(name="probs", bufs=5))
    i16_pool = ctx.enter_context(tc.tile_pool(name="i16p", bufs=3))
    outt_pool = ctx.enter_context(tc.tile_pool(name="outt", bufs=2))
    small = ctx.enter_context(tc.tile_pool(name="small", bufs=2))
    oslab_pool = ctx.enter_context(tc.tile_pool(name="oslab", bufs=3))

    # ---- constants / small inputs ----
    # mask -> per-k scale em = exp(-1e4 * (1 - mask)), [128, 32] (p, kchunk).
    # (host already folded 1/C into the DVE kc columns of the mask)
    mask_em = const.tile([128, 65], F32, name="mask_em")
    mask_t = mask_em[:, 0:32]
    em_sb = mask_em[:, 32:64]
    neg1e4 = mask_em[:, 64:65]
    nc.gpsimd.memset(neg1e4, -10000.0)
    nc.scalar.dma_start(mask_t, io["mask_pk"][:])
    nc.scalar.activation(em_sb, mask_t, AF.Exp, scale=10000.0, bias=neg1e4)

    # weights loaded contiguously (q | k | v along free dim)
    w_all = const.tile([128, NDC, 3 * GD], F16, name="w_all")
    for i, nm in ((1, "wk"), (0, "wq"), (2, "wv")):
        nc.scalar.dma_start(
            w_all[:, :, i * GD : (i + 1) * GD],
            io[nm].rearrange("(dc p) n -> p dc n", p=128),
        )
    wv_sb = w_all[:, :, 2 * GD : 3 * GD]

    # q/k weights with each head's 64 columns duplicated (projection then
    # replicates qT/kT on both partition halves at no extra PE cost)
    w_dup = const.tile([128, NDC, 2, HPG, 128], F16, name="w_dup")
    for i in (1, 0):
        for h in range(HPG):
            for rep in range(2):
                nc.vector.tensor_copy(
                    w_dup[:, :, i, h, rep * DK : (rep + 1) * DK],
                    w_all[:, :, i * GD + h * DK : i * GD + (h + 1) * DK],
                )

    bqbk = const.tile([128, 2 * HPG], F32, name="bqbk")
    nc.scalar.dma_start(bqbk[:], io["bqbk_pk"][:])

    bfpack = const.tile([1, 320], F16, name="bfpack")
    nc.gpsimd.memset(bfpack[:, 0:128], 1.0)
    nc.scalar.dma_start(bfpack[:, 128 : 128 + GD], io["bv_r"][:])
    ones_row = bfpack[:, 0:128]
    bv_sb = bfpack[:, 128 : 128 + GD]

    ident = const.tile([128, 128], F32, name="ident")
    make_identity(nc, ident[:])

    # ---- persistent projection outputs (qT/kT replicated on both halves) ----
    qT = proj.tile([128, HPG, S], BF16, name="qT")
    kT = proj.tile([128, HPG, S], BF16, name="kT")
    vE = proj.tile([128, NKC, HPG, DK + 1], F16, name="vE")
    nc.gpsimd.memset(vE[:], 1.0)  # ones col 64; data cols overwritten below

    # ---- per-quarter: transpose + project ----
    def load_xt_quarter(nm, qq):
        # host supplies x d-chunk-major [6*4096, 128] so each xbar transpose
        # reads a fully contiguous [1024, 128] block
        xt = xt_pool.tile([128, NDC, QTR], F16, tag="xt", name=f"xt_{nm}_{qq}")
        for dc in range(NDC):
            base = dc * S + qq * QTR
            nc.sync.dma_start(
                out=xt[:, dc, :], in_=io[nm][base : base + QTR, :],
                transpose=True,
            )
        return xt

    def proj_qk(xt, qq, wi, bias, dst):
        for h in range(HPG):
            for sqq in range(SQQ):
                sq = qq * SQQ + sqq
                ps = scores_pool.tile(
                    [128, SQ], F32, tag="scores", name=f"ps_{qq}_{wi}_{h}_{sqq}"
                )
                for dc in range(NDC):
                    nc.tensor.matmul(
                        ps[:],
                        lhsT=w_dup[:, dc, wi, h, :],
                        rhs=xt[:, dc, sqq * SQ : (sqq + 1) * SQ],
                        start=(dc == 0),
                        stop=(dc == NDC - 1),
                    )
                nc.vector.tensor_scalar(
                    dst[:, h, sq * SQ : (sq + 1) * SQ], ps[:],
                    bias[:, h : h + 1], None, ALU.add,
                )

    def proj_v(xt, qq):
        for scq in range(SCQ):
            sc = qq * SCQ + scq
            ps = scores_pool.tile([128, GD], F32, tag="scores", name=f"psv_{qq}_{scq}")
            for dc in range(NDC):
                nc.tensor.matmul(
                    ps[:],
                    lhsT=xt[:, dc, scq * 128 : (scq + 1) * 128],
                    rhs=wv_sb[:, dc, :],
                    start=(dc == 0),
                    stop=False,
                )
            nc.tensor.matmul(
                ps[:], lhsT=ones_row[:, 0:128], rhs=bv_sb[:], start=False, stop=True
            )
            for h in range(HPG):
                nc.vector.tensor_copy(
                    vE[:, sc, h, 0:DK], ps[:, h * DK : (h + 1) * DK]
                )
            # fold mask scale (and the DVE 1/C correction) into v + ones col
            nc.vector.tensor_scalar(
                vE[:, sc, :, :], vE[:, sc, :, :], em_sb[:, sc : sc + 1], None,
                ALU.mult,
            )

    def proj_kv_quarter(qq):
        xt_k = load_xt_quarter("xk", qq)
        proj_qk(xt_k, qq, 1, bqbk[:, HPG : 2 * HPG], kT)
        xt_v = load_xt_quarter("xv", qq)
        proj_v(xt_v, qq)

    def proj_q_group(xt, qq, h, sqq):
        sq = qq * SQQ + sqq
        ps = scores_pool.tile([128, SQ], F32, tag="scores", name=f"psq_{qq}_{h}_{sqq}")
        for dc in range(NDC):
            nc.tensor.matmul(
                ps[:],
                lhsT=w_dup[:, dc, 0, h, :],
                rhs=xt[:, dc, sqq * SQ : (sqq + 1) * SQ],
                start=(dc == 0),
                stop=(dc == NDC - 1),
            )
        nc.vector.tensor_scalar(
            qT[:, h, sq * SQ : (sq + 1) * SQ], ps[:],
            bqbk[:, h : h + 1], None, ALU.add,
        )

    # ---- attention ----
    pending = [None]  # finalize closure for the previous (h, sq)

    def make_finalize(pv, h, sq):
        def fin():
            ot = outt_pool.tile([DK + 1, 2 * SQ], F32, tag="outt", name=f"ot_{h}_{sq}")
            nc.vector.tensor_copy(ot[:], pv[:])
            tr = scores_pool.tile(
                [128, 4 * (DK + 1)], F32, tag="scores", name=f"tr_{h}_{sq}"
            )
            for t in range(4):
                # accumulate lo-strip and hi-strip transposes into the same
                # PSUM slice: merges the split-PV halves for free
                nc.tensor.matmul(
                    tr[:, t * (DK + 1) : (t + 1) * (DK + 1)],
                    lhsT=ot[:, t * 128 : (t + 1) * 128],
                    rhs=ident[0 : DK + 1, 0 : DK + 1],
                    is_transpose=True,
                    start=True,
                    stop=False,
                )
                nc.tensor.matmul(
                    tr[:, t * (DK + 1) : (t + 1) * (DK + 1)],
                    lhsT=ot[:, SQ + t * 128 : SQ + (t + 1) * 128],
                    rhs=ident[0 : DK + 1, 0 : DK + 1],
                    is_transpose=True,
                    start=False,
                    stop=True,
                )
            rc = small.tile([128, 4], F32, tag="recip", name=f"rc_{h}_{sq}")
            osl = oslab_pool.tile([128, 4, DK], F32, tag="oslab", name=f"os_{h}_{sq}")
            for t in range(4):
                nc.vector.reciprocal(
                    rc[:, t : t + 1], tr[:, t * (DK + 1) + DK : t * (DK + 1) + DK + 1]
                )
                nc.vector.tensor_scalar(
                    osl[:, t, :],
                    tr[:, t * (DK + 1) : t * (DK + 1) + DK],
                    rc[:, t : t + 1],
                    None,
                    ALU.mult,
                )
            nc.gpsimd.dma_start(
                out=io["out"].rearrange(
                    "(sq t p) n -> sq p t n", sq=NSQ, t=4, p=128
                )[sq, :, :, h * DK : (h + 1) * DK],
                in_=osl[:],
            )
        return fin

    def emit_pv(pv, h, p0, ppr):
        for j in range(GRP):
            kc = p0 + j
            nc.tensor.matmul(
                pv[:, 0, :],
                lhsT=vE[0:DK, kc, h, :],
                rhs=ppr[0:DK, j, :],
                start=(kc == 0),
                stop=(kc == NKC - 1),
            )
            nc.tensor.matmul(
                pv[:, 1, :],
                lhsT=vE[DK:128, kc, h, :],
                rhs=ppr[DK:128, j, :],
                start=(kc == 0),
                stop=(kc == NKC - 1),
            )

    def attention_gen():
        for h in range(HPG):
            for sq in range(NSQ):
                pv = pv_pool.tile([DK + 1, 2, SQ], F32, tag="pv", name=f"pv_{h}_{sq}")
                ready = []  # (kc0, probs tile) groups awaiting PV emission
                for gi in range(NG):
                    kc0 = gi * GRP
                    # PV first so the PE queue has useful work in front of
                    # the next slot-wait
                    if len(ready) >= 2:
                        p0, ppr = ready.pop(0)
                        emit_pv(pv, h, p0, ppr)
                    sc_t = scores_pool.tile(
                        [128, GRP, SQ], F32, tag="scores", name=f"sc_{h}_{sq}_{gi}"
                    )
                    # row-tiled concurrent pair: even kc on rows 0:64, odd on
                    # rows 64:128 (duplicated halves of kT/qT)
                    nc.tensor.matmul(
                        sc_t[:, 0, :],
                        lhsT=kT[0:DK, h, kc0 * KCW : (kc0 + 1) * KCW],
                        rhs=qT[0:DK, h, sq * SQ : (sq + 1) * SQ],
                        start=True,
                        stop=True,
                    )
                    nc.tensor.matmul(
                        sc_t[:, 1, :],
                        lhsT=kT[DK:128, h, (kc0 + 1) * KCW : (kc0 + 2) * KCW],
                        rhs=qT[DK:128, h, sq * SQ : (sq + 1) * SQ],
                        start=True,
                        stop=True,
                    )
                    pr = probs_pool.tile(
                        [128, GRP, SQ], F16, tag="probs", name=f"pr_{h}_{sq}_{gi}"
                    )
                    if gi in dve_gi:
                        i16 = i16_pool.tile(
                            [128, 2, GRP * SQ], I16, tag="i16", name=f"i_{h}_{sq}_{gi}"
                        )
                        i1 = i16[:, 0, :]
                        i2 = i16[:, 1, :]
                        nc.vector.tensor_scalar(
                            i1, sc_t[:, :, :], A_EXP, B1_EXP, ALU.mult, ALU.add
                        )
                        nc.vector.tensor_scalar(i2, i1, 512, None, ALU.subtract)
                        nc.vector.tensor_tensor(
                            pr[:, :, :].bitcast(F16),
                            i1.bitcast(F16),
                            i2.bitcast(F16),
                            ALU.mult,
                        )
                    else:
                        nc.scalar.activation(pr[:], sc_t[:], AF.Exp, scale=0.125)
                    ready.append((kc0, pr))
                    if gi == 1 and pending[0] is not None:
                        pending[0]()
                        pending[0] = None
                    yield (h, sq, gi)
                # drain remaining groups (their exps are queued/just done)
                for p0, ppr in ready:
                    emit_pv(pv, h, p0, ppr)
                pending[0] = make_finalize(pv, h, sq)

        pending[0]()

    # Interleave k/v projection quarters with the first attention iteration's
    # k-chunk groups: group gi covers kc 2gi..2gi+1, requiring k/v quarters up
    # to (2gi+1)//8; the first iteration uses q chunk sq=0 (quarter 0).
    gen = attention_gen()

    def advance(n):
        for _ in range(n):
            if next(gen, None) is None:
                break

    # quarter 0 in k, q, v order: the first QK group needs kT+qT only (the
    # first PV trails by two exp groups, so v can land a little later)
    xt_k0 = load_xt_quarter("xk", 0)
    proj_qk(xt_k0, 0, 1, bqbk[:, HPG : 2 * HPG], kT)
    xt_q0 = load_xt_quarter("xq", 0)
    for h in range(HPG):
        for sqq in range(SQQ):
            proj_q_group(xt_q0, 0, h, sqq)
    advance(1)       # g0: kc 0..1 (needs only kT+qT of quarter 0; no PV yet)
    xt_v0 = load_xt_quarter("xv", 0)
    proj_v(xt_v0, 0)
    advance(3)       # g1..g3: kc 2..7 (first PV -- needing vE -- fires at g2)
    proj_kv_quarter(1)
    xt_q1 = load_xt_quarter("xq", 1)
    for h in range(HPG):
        for sqq in range(SQQ):
            proj_q_group(xt_q1, 1, h, sqq)
    advance(4)       # g4..g7: kc 8..15 (quarter 1)
    proj_kv_quarter(2)
    xt_q2 = load_xt_quarter("xq", 2)
    for h in range(HPG):
        for sqq in range(SQQ):
            proj_q_group(xt_q2, 2, h, sqq)
    advance(4)       # g8..g11: kc 16..23 (quarter 2)
    proj_kv_quarter(3)
    xt_q3 = load_xt_quarter("xq", 3)
    for h in range(HPG):
        for sqq in range(SQQ):
            proj_q_group(xt_q3, 3, h, sqq)
    for _ in gen:
        pass


def _build():
    nc = bacc.Bacc("TRN2", target_bir_lowering=False, debug=False)
    io = {}
    for nm, shape, dt in (
        ("xq", [NDC * S, 128], F16), ("xk", [NDC * S, 128], F16),
        ("xv", [NDC * S, 128], F16),
        ("wq", [D, GD], F16), ("wk", [D, GD], F16), ("wv", [D, GD], F16),
        ("bqbk_pk", [128, 2 * HPG], F32),
        ("bv_r", [1, GD], F16), ("mask_pk", [128, NKC], F32),
    ):
        io[nm] = nc.dram_tensor(nm, shape, dt, kind="ExternalInput").ap()
    io["out"] = nc.dram_tensor("out", [S, GD], F32, kind="ExternalOutput").ap()

    dup = int(os.environ.get("BASS_DUP", "1"))
    with tile.TileContext(nc) as tc:
        for _ in range(dup):
            with ExitStack() as ctx:
                _emit(ctx, tc, io)
    nc.compile()
    return nc


_NC = None


def _get_nc():
    global _NC
    if _NC is None:
        _NC = _build()
    return _NC


def make_in_maps(query, key, value, mask, Wq, bq, Wk, bk, Wv, bv):
    bf = lambda a: np.ascontiguousarray(a).astype(F16_NP)
    bf3 = lambda a: np.ascontiguousarray(
        np.asarray(a).reshape(S, NDC, 128).transpose(1, 0, 2).reshape(NDC * S, 128)
    ).astype(F16_NP)
    f32 = lambda a: np.ascontiguousarray(np.asarray(a, np.float32))
    # fold the DVE-group exp scale 1/C into the mask columns for those kc's:
    # em = exp(-1e4*(1 - m)); m' = m - ln(C)/1e4  =>  em' = em / C
    dve_kc = sorted(
        kc for g in dve_gi_set() for kc in (GRP * g, GRP * g + 1)
    )
    in_maps = []
    for c in range(N_CORES):
        b, g = divmod(c, 4)
        cols = slice(g * GD, (g + 1) * GD)
        mask_pk = f32(np.asarray(mask)[b].reshape(NKC, 128).T)
        if dve_kc:
            mask_pk[:, dve_kc] -= np.float32(np.log(C_EXP) / 1e4)
        in_maps.append({
            "xq": bf3(query[b]),
            "xk": bf3(key[b]),
            "xv": bf3(value[b]),
            "wq": bf(Wq[:, cols]),
            "wk": bf(Wk[:, cols]),
            "wv": bf(Wv[:, cols]),
            "bqbk_pk": f32(np.tile(np.concatenate(
                [np.asarray(bq)[cols].reshape(HPG, DK).T,
                 np.asarray(bk)[cols].reshape(HPG, DK).T], axis=1), (2, 1))),
            "bv_r": bf(np.asarray(bv)[cols].reshape(1, GD)),
            "mask_pk": mask_pk,
        })
    return in_maps


def kernel(query, key, value, mask, Wq, bq, Wk, bk, Wv, bv):
    query = np.asarray(query, np.float32)
    key = np.asarray(key, np.float32)
    value = np.asarray(value, np.float32)
    nc = _get_nc()
    in_maps = make_in_maps(query, key, value, mask, Wq, bq, Wk, bk, Wv, bv)
    res = run_bass_kernel_spmd(nc, in_maps, core_ids=list(range(N_CORES)))
    out = np.empty((B, S, D), np.float32)
    for c in range(N_CORES):
        b, g = divmod(c, 4)
        out[b, :, g * GD : (g + 1) * GD] = res.results[c]["out"]
    return out
